# revision 18
# baseline (speedup 1.0000x reference)
"""Trainium2 Bass kernel for nn_DowngradeProtocol (mtf blur + fineshift + 4x decimate).

F9 restructure (stage A: image-stationary matmuls produce 9 mtf-column planes
t_v[z, y']; stage B contracts them with v-shifted horizontal banded kernels).

v5 = v1 sequential skeleton (tensor-bound, copies fully hidden) plus:
  - BV dedup: interior row-windows g=1..8 share one banded matrix (the
    vertical kernel is shift-invariant away from the image edge): per-image
    vertical weights are 3 variants instead of 10; window g=9 has only 13
    live output rows and is packed to 117 moving columns (-0.5us/img PE).
  - PSUM banks hold a window pair in v-major layout [v, gi, n] so each
    bank drains with one 2-free-dim copy.
  - DMA queue split: bv + image windows + output stores on the SP HWDGE
    queue (bv first, 3 window chunks so the first matmul gates on 0.7MB,
    not 3.2MB); kh on the Activation HWDGE queue (one ~0.6us trigger per
    image on the scalar engine).  All load tiles have one ring slot per
    image so no DMA trigger ever blocks an engine stream.
"""
import sys

import numpy as np

for _p in ("/opt/trn_rl_repo",):
    if _p not in sys.path:
        sys.path.insert(0, _p)

# ---------------------------------------------------------------- constants
H = W = 1024
OUT = 256
NG = 10       # row windows of 128 (stride 108); 27 sampled rows each
GRP = 27
NV = 9        # mtf horizontal offsets
N_CORES = 8
IMG_PER_CORE = 4
BVW = 243 + 243 + 117   # g0 variant, interior variant, g9 packed variant

_HALF = np.asarray([0.5, 0.305334091185, 0, -0.072698593239, 0, 0.021809577942,
                    0, -0.005192756653, 0, 0.000807762146, 0, -6.0081482e-05]) * 2.0
_FULL23 = np.concatenate([_HALF[1:][::-1], _HALF])
F12 = _FULL23[::2]
DELTA12 = np.zeros(12)
DELTA12[6] = 1.0


def _wb(g):
    return 108 * g - 10


# ------------------------------------------------------- host weight builders
def build_BV(m2d, rr):
    """Vertical banded matrices [NG, 9, 128, GRP] f64 for one image.

    BV[g, v, k, n] = weight of input row (wb(g)+k) for sampled output row
    y'=27g+n under mtf column offset v, with the fine-shift vertical kernel
    fused in, input-edge replication folded (row clip), and fine-stage
    zero-padding honored.  g=1..8 are identical (no edge effects)."""
    ri, rf = rr // 2, rr % 2
    f = F12 if rf == 1 else DELTA12
    BV = np.zeros((NG, 9, 128, GRP))
    for g in range(NG):
        wb = _wb(g)
        for n in range(GRP):
            yp = 27 * g + n
            if yp >= OUT:
                continue
            Ry = 2 + 4 * yp - ri
            for up in range(12):
                i1 = Ry + up - 6
                fw = f[up]
                if fw == 0.0 or not (0 <= i1 < H):
                    continue
                for u in range(9):
                    k = min(max(i1 + u - 4, 0), H - 1)
                    BV[g, :, k - wb, n] += fw * m2d[u, :]
    return BV


def _khv_geometry():
    """Static stage-B block table: for each (v, tile t) the x'-range whose
    12-tap horizontal band (shifted by v, edge-clipped) intersects z-column
    tile t, unioned over ci in {0,1,2} so the program is image-independent.
    Returns (blocks=[(v, t, x0, x1, koff)], KHW)."""
    nz = np.zeros((NV, 8, OUT), bool)
    for ci in range(3):
        for x in range(OUT):
            Cx = 2 + 4 * x - ci
            for tt in range(12):
                jz = Cx + tt - 6
                if 0 <= jz < W:
                    for v in range(NV):
                        jx = min(max(jz + v - 4, 0), W - 1)
                        nz[v, jx // 128, x] = True
    blocks = []
    off = 0
    for v in range(NV):
        for t in range(8):
            xs = np.nonzero(nz[v, t])[0]
            if len(xs) == 0:
                continue
            x0, x1 = int(xs[0]), int(xs[-1]) + 1
            assert np.all(nz[v, t, x0:x1]), (v, t)
            blocks.append((v, t, x0, x1, off))
            off += x1 - x0
    return blocks, off


BLOCKS, KHW = _khv_geometry()
WCOLS = BVW + KHW


def build_KHV(cc_val):
    """Per-image stage-B data [128, KHW] f64 filled into the static blocks."""
    ci, cf = cc_val // 2, cc_val % 2
    h = F12 if cf == 1 else DELTA12
    F = np.zeros((NV, W, OUT))
    for x in range(OUT):
        Cx = 2 + 4 * x - ci
        for tt in range(12):
            jz = Cx + tt - 6
            if not (0 <= jz < W):
                continue
            hv = h[tt]
            if hv == 0.0:
                continue
            for v in range(NV):
                jx = min(max(jz + v - 4, 0), W - 1)
                F[v, jx, x] += hv
    kh = np.zeros((128, KHW))
    for (v, t, x0, x1, off) in BLOCKS:
        kh[:, off:off + (x1 - x0)] = F[v, 128 * t:128 * (t + 1), x0:x1]
    return kh


# ------------------------------------------------------------- bass program
_PROGRAM = None


def _split_multi_waits(nc):
    """This container's walrus codegen allows only ONE sync-wait per
    instruction; hoist extra waits onto NoOps inserted just before, on the
    same engine (engine blocks on each in program order — semantics kept)."""
    import concourse.mybir as mybir

    n_split = 0
    for fn in nc.m.functions:
        for bb in fn.blocks:
            out = []
            changed = False
            for inst in bb.instructions:
                si = getattr(inst, "sync_info", None)
                waits = list(si.on_wait) if si is not None and si.on_wait else []
                if len(waits) > 1:
                    for w in waits[:-1]:
                        nop = mybir.InstNoOp(
                            text_hint="wait_split",
                            name=f"I-{nc.next_id()}",
                            engine=inst.engine,
                            ins=[], outs=[],
                            sync_info=mybir.SyncInfo(on_wait=[w], on_update=[]),
                        )
                        nc.register_instruction(nop)
                        out.append(nop)
                        n_split += 1
                    si.on_wait[:] = waits[-1:]
                    changed = True
                out.append(inst)
            if changed:
                bb.instructions[:] = out
    return n_split


def _build_program():
    import concourse.bass as bass
    import concourse.mybir as mybir
    from concourse.tile import TileContext

    f32, f16 = mybir.dt.float32, mybir.dt.float16
    nc = bass.Bass(target_bir_lowering=False, trn_type="TRN2")

    x_in = nc.dram_tensor("x", [IMG_PER_CORE, 128, NG, W], f16,
                          kind="ExternalInput")
    w_in = nc.dram_tensor("w", [128, IMG_PER_CORE * WCOLS], f16,
                          kind="ExternalInput")
    out_t = nc.dram_tensor("out", [IMG_PER_CORE, OUT, OUT], f16,
                           kind="ExternalOutput")

    with TileContext(nc) as tc:
        with (
            tc.tile_pool(name="pw", bufs=4) as pw,
            tc.tile_pool(name="pxe", bufs=4) as pxe,
            tc.tile_pool(name="pt", bufs=2) as pt,
            tc.tile_pool(name="pout", bufs=3) as pout,
            tc.tile_pool(name="psA", bufs=6, space="PSUM") as psA,
            tc.tile_pool(name="psB", bufs=2, space="PSUM") as psB,
        ):
            zt = pw.tile([128, OUT], f16, tag="zt", bufs=1)
            nc.vector.memset(zt[:, :], 0.0)
            # warm-up: PE work that depends only on the memset, filling the
            # initial DMA-fill idle and ramping the p-state before real data
            # arrives (results never read).
            wu = psB.tile([128, OUT], f32, tag="psB", name="wu")
            for _ in range(24):
                nc.tensor.matmul(wu[:, :], lhsT=zt[:, 0:128], rhs=zt[:, :],
                                 start=True, stop=True)

            ncopy = 0

            def copy(dst, src):
                # GPSIMD cannot read PSUM, so only DVE + Activation rotate.
                nonlocal ncopy
                if ncopy % 2 == 0:
                    nc.vector.tensor_copy(out=dst, in_=src)
                else:
                    nc.scalar.copy(out=dst, in_=src)
                ncopy += 1

            # ---- all load triggers up front.  Each tile tag has one ring
            # slot per image, so no trigger blocks; keeping them ahead of
            # the out-store triggers in the SP stream means image i+1's
            # loads don't queue behind stores that wait on compute.
            tiles = {}
            for img in range(IMG_PER_CORE):
                woff = img * WCOLS
                w_sb = pw.tile([128, WCOLS], f16, tag="w", name="w_sb")
                nc.sync.dma_start(out=w_sb[:, 0:BVW],
                                  in_=w_in[:, woff:woff + BVW])
                xa = pxe.tile([128, 2, W], f16, tag="xa", name="xa")
                nc.sync.dma_start(out=xa[:, :, :], in_=x_in[img, :, 0:2, :])
                xb = pxe.tile([128, 4, W], f16, tag="xb", name="xb")
                nc.sync.dma_start(out=xb[:, :, :], in_=x_in[img, :, 2:6, :])
                xc = pxe.tile([128, 4, W], f16, tag="xc", name="xc")
                nc.sync.dma_start(out=xc[:, :, :], in_=x_in[img, :, 6:NG, :])
                nc.scalar.dma_start(out=w_sb[:, BVW:WCOLS],
                                    in_=w_in[:, woff + BVW:woff + WCOLS])
                tiles[img] = (w_sb, xa, xb, xc)

            for img in range(IMG_PER_CORE):
                w_sb, xa, xb, xc = tiles[img]

                def win(g):
                    if g < 2:
                        return xa[:, g]
                    if g < 6:
                        return xb[:, g - 2]
                    return xc[:, g - 6]

                # ---- stage A.  One PSUM bank per window pair, v-major
                # layout [v, gi, n]; y' cols 256..269 of tpl get garbage
                # from g9's unwritten PSUM tail (never read by stage B).
                tpl = {cc: pt.tile([128, NV, 270], f16, tag=f"T{cc}",
                                   name=f"T{cc}")
                       for cc in range(8)}
                for gp in range(5):
                    if img == 0 and gp == 3:
                        # image-0's xc windows are still in flight (the DMA
                        # engine ramps from ~110 to ~440 GB/s over the first
                        # 10us); keep the PE hot instead of idling.
                        for _ in range(12):
                            nc.tensor.matmul(wu[:, :], lhsT=zt[:, 0:128],
                                             rhs=zt[:, :], start=True,
                                             stop=True)
                    for cc in range(8):
                        ps = psA.tile([128, 512], f32, tag="psA", name="ps")
                        r = ps[:, 0:486].rearrange(
                            "P (v gi n) -> P v gi n", v=NV, gi=2, n=GRP)
                        if gp < 4:
                            for gi in range(2):
                                g = 2 * gp + gi
                                rhs = (w_sb[:, 0:243] if g == 0
                                       else w_sb[:, 243:486])
                                nc.tensor.matmul(
                                    r[:, :, gi, :],
                                    lhsT=win(g)[:, 128 * cc:128 * (cc + 1)],
                                    rhs=rhs, start=True, stop=True)
                        else:
                            nc.tensor.matmul(
                                r[:, :, 0, :],
                                lhsT=win(8)[:, 128 * cc:128 * (cc + 1)],
                                rhs=w_sb[:, 243:486], start=True, stop=True)
                            nc.tensor.matmul(
                                r[:, :, 1, 0:13],
                                lhsT=win(9)[:, 128 * cc:128 * (cc + 1)],
                                rhs=w_sb[:, 486:603], start=True, stop=True)
                        src = ps[:, 0:486].rearrange(
                            "P (v gin) -> P v gin", v=NV, gin=2 * GRP)
                        copy(tpl[cc][:, :, 54 * gp:54 * gp + 54], src)

                # ---- stage B: accumulate all (tile, v) blocks into out PSUM
                for yc in range(2):
                    po = psB.tile([128, OUT], f32, tag="psB")
                    # zero + set PSUM written-bits via an all-zero matmul;
                    # streams the static zero tile so it can issue early.
                    nc.tensor.matmul(
                        po[:, :], lhsT=zt[:, 0:128], rhs=zt[:, :],
                        start=True, stop=False, skip_group_check=True)
                    for bi, (v, t, x0, x1, koff) in enumerate(BLOCKS):
                        nc.tensor.matmul(
                            po[:, x0:x1],
                            lhsT=tpl[t][:, v, 128 * yc:128 * yc + 128],
                            rhs=w_sb[:, BVW + koff:BVW + koff + (x1 - x0)],
                            start=False, stop=(bi == len(BLOCKS) - 1),
                            skip_group_check=True)
                    ot = pout.tile([128, OUT], f16, tag="ot")
                    copy(ot[:, :], po[:, :])
                    nc.sync.dma_start(
                        out=out_t[img, 128 * yc:128 * (yc + 1), :],
                        in_=ot[:, :])
    _split_multi_waits(nc)
    return nc


def _get_program():
    global _PROGRAM
    if _PROGRAM is None:
        _PROGRAM = _build_program()
    return _PROGRAM


# ------------------------------------------------------------------ kernel
def _window_image(img16):
    """[NG, 128, W] f16 pre-extracted row windows; out-of-image partitions
    get arbitrary finite rows (their BV weights are exactly zero)."""
    xw = np.empty((NG, 128, W), np.float16)
    for g in range(NG):
        wb = _wb(g)
        r0, r1 = max(0, wb), min(H, wb + 128)
        p0, pn = r0 - wb, r1 - r0
        xw[g, p0:p0 + pn] = img16[r0:r1]
        if p0 > 0:
            xw[g, 0:p0] = img16[0:p0]
        if p0 + pn < 128:
            xw[g, p0 + pn:] = img16[0:128 - (p0 + pn)]
    return xw


def _make_in_maps(outputs, mtf, r, c):
    outputs = np.ascontiguousarray(outputs, np.float32)
    mtf64 = np.asarray(mtf, np.float64)
    in_maps = []
    for core in range(N_CORES):
        xs = np.empty((IMG_PER_CORE, 128, NG, W), np.float16)
        w = np.empty((128, IMG_PER_CORE, WCOLS), np.float16)
        for i in range(IMG_PER_CORE):
            p = core * IMG_PER_CORE + i
            b, ch = divmod(p, 8)
            xs[i] = _window_image(
                outputs[b, ch].astype(np.float16)).transpose(1, 0, 2)
            BV = build_BV(mtf64[:, :, ch], int(r[b, ch]))   # [NG,9,128,GRP]
            w[:, i, 0:243] = BV[0].transpose(1, 0, 2).reshape(
                128, 243).astype(np.float16)
            w[:, i, 243:486] = BV[1].transpose(1, 0, 2).reshape(
                128, 243).astype(np.float16)
            w[:, i, 486:603] = BV[9][:, :, :13].transpose(1, 0, 2).reshape(
                128, 117).astype(np.float16)
            w[:, i, BVW:] = build_KHV(int(c[b, ch])).astype(np.float16)
        in_maps.append({
            "x": xs,
            "w": np.ascontiguousarray(w.reshape(128, -1)),
        })
    return in_maps


def run(outputs, mtf, r, c, trace=False, trace_cores=None):
    from concourse.bass_utils import run_bass_kernel_spmd

    nc = _get_program()
    in_maps = _make_in_maps(outputs, mtf, r, c)
    res = run_bass_kernel_spmd(nc, in_maps, core_ids=list(range(N_CORES)),
                               trace=trace, trace_cores=trace_cores)
    full = np.empty((4, 8, OUT, OUT), np.float32)
    for core in range(N_CORES):
        o = np.asarray(res.results[core]["out"])
        for i in range(IMG_PER_CORE):
            p = core * IMG_PER_CORE + i
            b, ch = divmod(p, 8)
            full[b, ch] = o[i].astype(np.float32)
    return full, res


def kernel(outputs, mtf, r, c):
    full, _ = run(outputs, mtf, r, c)
    return full


# revision 20
# speedup vs baseline: 1.0442x; 1.0442x over previous
"""Trainium2 Bass kernel for nn_DowngradeProtocol (mtf blur + fineshift + 4x decimate).

F9 restructure (stage A: image-stationary matmuls produce 9 mtf-column planes
t_v[z, y']; stage B contracts them with v-shifted horizontal banded kernels).

v5 = v1 sequential skeleton (tensor-bound, copies fully hidden) plus:
  - BV dedup: interior row-windows g=1..8 share one banded matrix (the
    vertical kernel is shift-invariant away from the image edge): per-image
    vertical weights are 3 variants instead of 10; window g=9 has only 13
    live output rows and is packed to 117 moving columns (-0.5us/img PE).
  - PSUM banks hold a window pair in v-major layout [v, gi, n] so each
    bank drains with one 2-free-dim copy.
  - DMA queue split: bv + image windows + output stores on the SP HWDGE
    queue (bv first, 3 window chunks so the first matmul gates on 0.7MB,
    not 3.2MB); kh on the Activation HWDGE queue (one ~0.6us trigger per
    image on the scalar engine).  All load tiles have one ring slot per
    image so no DMA trigger ever blocks an engine stream.
"""
import sys

import numpy as np

for _p in ("/opt/trn_rl_repo",):
    if _p not in sys.path:
        sys.path.insert(0, _p)

# ---------------------------------------------------------------- constants
H = W = 1024
OUT = 256
NG = 10       # row windows of 128 (stride 108); 27 sampled rows each
GRP = 27
NV = 9        # mtf horizontal offsets
N_CORES = 8
IMG_PER_CORE = 4
BVW = 243 + 243 + 117   # g0 variant, interior variant, g9 packed variant

_HALF = np.asarray([0.5, 0.305334091185, 0, -0.072698593239, 0, 0.021809577942,
                    0, -0.005192756653, 0, 0.000807762146, 0, -6.0081482e-05]) * 2.0
_FULL23 = np.concatenate([_HALF[1:][::-1], _HALF])
F12 = _FULL23[::2]
DELTA12 = np.zeros(12)
DELTA12[6] = 1.0


def _wb(g):
    return 108 * g - 10


# ------------------------------------------------------- host weight builders
def build_BV(m2d, rr):
    """Vertical banded matrices [NG, 9, 128, GRP] f64 for one image.

    BV[g, v, k, n] = weight of input row (wb(g)+k) for sampled output row
    y'=27g+n under mtf column offset v, with the fine-shift vertical kernel
    fused in, input-edge replication folded (row clip), and fine-stage
    zero-padding honored.  g=1..8 are identical (no edge effects)."""
    ri, rf = rr // 2, rr % 2
    f = F12 if rf == 1 else DELTA12
    BV = np.zeros((NG, 9, 128, GRP))
    for g in range(NG):
        wb = _wb(g)
        for n in range(GRP):
            yp = 27 * g + n
            if yp >= OUT:
                continue
            Ry = 2 + 4 * yp - ri
            for up in range(12):
                i1 = Ry + up - 6
                fw = f[up]
                if fw == 0.0 or not (0 <= i1 < H):
                    continue
                for u in range(9):
                    k = min(max(i1 + u - 4, 0), H - 1)
                    BV[g, :, k - wb, n] += fw * m2d[u, :]
    return BV


def _khv_geometry():
    """Static stage-B block table: for each (v, tile t) the x'-range whose
    12-tap horizontal band (shifted by v, edge-clipped) intersects z-column
    tile t, unioned over ci in {0,1,2} so the program is image-independent.
    Returns (blocks=[(v, t, x0, x1, koff)], KHW)."""
    nz = np.zeros((NV, 8, OUT), bool)
    for ci in range(3):
        for x in range(OUT):
            Cx = 2 + 4 * x - ci
            for tt in range(12):
                jz = Cx + tt - 6
                if 0 <= jz < W:
                    for v in range(NV):
                        jx = min(max(jz + v - 4, 0), W - 1)
                        nz[v, jx // 128, x] = True
    blocks = []
    off = 0
    for v in range(NV):
        for t in range(8):
            xs = np.nonzero(nz[v, t])[0]
            if len(xs) == 0:
                continue
            x0, x1 = int(xs[0]), int(xs[-1]) + 1
            assert np.all(nz[v, t, x0:x1]), (v, t)
            blocks.append((v, t, x0, x1, off))
            off += x1 - x0
    return blocks, off


BLOCKS, KHW = _khv_geometry()
WCOLS = BVW + KHW


def build_KHV(cc_val):
    """Per-image stage-B data [128, KHW] f64 filled into the static blocks."""
    ci, cf = cc_val // 2, cc_val % 2
    h = F12 if cf == 1 else DELTA12
    F = np.zeros((NV, W, OUT))
    for x in range(OUT):
        Cx = 2 + 4 * x - ci
        for tt in range(12):
            jz = Cx + tt - 6
            if not (0 <= jz < W):
                continue
            hv = h[tt]
            if hv == 0.0:
                continue
            for v in range(NV):
                jx = min(max(jz + v - 4, 0), W - 1)
                F[v, jx, x] += hv
    kh = np.zeros((128, KHW))
    for (v, t, x0, x1, off) in BLOCKS:
        kh[:, off:off + (x1 - x0)] = F[v, 128 * t:128 * (t + 1), x0:x1]
    return kh


# ------------------------------------------------------------- bass program
_PROGRAM = None


def _split_multi_waits(nc):
    """This container's walrus codegen allows only ONE sync-wait per
    instruction; hoist extra waits onto NoOps inserted just before, on the
    same engine (engine blocks on each in program order — semantics kept)."""
    import concourse.mybir as mybir

    n_split = 0
    for fn in nc.m.functions:
        for bb in fn.blocks:
            out = []
            changed = False
            for inst in bb.instructions:
                si = getattr(inst, "sync_info", None)
                waits = list(si.on_wait) if si is not None and si.on_wait else []
                if len(waits) > 1:
                    for w in waits[:-1]:
                        nop = mybir.InstNoOp(
                            text_hint="wait_split",
                            name=f"I-{nc.next_id()}",
                            engine=inst.engine,
                            ins=[], outs=[],
                            sync_info=mybir.SyncInfo(on_wait=[w], on_update=[]),
                        )
                        nc.register_instruction(nop)
                        out.append(nop)
                        n_split += 1
                    si.on_wait[:] = waits[-1:]
                    changed = True
                out.append(inst)
            if changed:
                bb.instructions[:] = out
    return n_split


def _build_program():
    import concourse.bass as bass
    import concourse.mybir as mybir
    from concourse.tile import TileContext

    f32, f16 = mybir.dt.float32, mybir.dt.float16
    nc = bass.Bass(target_bir_lowering=False, trn_type="TRN2")

    x_in = nc.dram_tensor("x", [IMG_PER_CORE, 128, NG, W], f16,
                          kind="ExternalInput")
    w_in = nc.dram_tensor("w", [128, IMG_PER_CORE * WCOLS], f16,
                          kind="ExternalInput")
    out_t = nc.dram_tensor("out", [IMG_PER_CORE, OUT, OUT], f16,
                           kind="ExternalOutput")

    with TileContext(nc) as tc:
        with (
            tc.tile_pool(name="pw", bufs=4) as pw,
            tc.tile_pool(name="pxe", bufs=4) as pxe,
            tc.tile_pool(name="pt", bufs=2) as pt,
            tc.tile_pool(name="pout", bufs=3) as pout,
            tc.tile_pool(name="psA", bufs=6, space="PSUM") as psA,
            tc.tile_pool(name="psB", bufs=2, space="PSUM") as psB,
        ):
            zt = pw.tile([128, OUT], f16, tag="zt", bufs=1)
            nc.vector.memset(zt[:, :], 0.0)
            # warm-up: PE work that depends only on the memset, filling the
            # initial DMA-fill idle and ramping the p-state before real data
            # arrives (results never read).
            wu = psB.tile([128, OUT], f32, tag="psB", name="wu")
            for _ in range(24):
                nc.tensor.matmul(wu[:, :], lhsT=zt[:, 0:128], rhs=zt[:, :],
                                 start=True, stop=True)

            ncopy = 0

            def copy(dst, src):
                # GPSIMD cannot read PSUM, so only DVE + Activation rotate.
                nonlocal ncopy
                if ncopy % 2 == 0:
                    nc.vector.tensor_copy(out=dst, in_=src)
                else:
                    nc.scalar.copy(out=dst, in_=src)
                ncopy += 1

            # ---- all load triggers up front.  Each tile tag has one ring
            # slot per image, so no trigger blocks; keeping them ahead of
            # the out-store triggers in the SP stream means image i+1's
            # loads don't queue behind stores that wait on compute.
            tiles = {}
            for img in range(IMG_PER_CORE):
                woff = img * WCOLS
                w_sb = pw.tile([128, WCOLS], f16, tag="w", name="w_sb")
                nc.sync.dma_start(out=w_sb[:, 0:BVW],
                                  in_=w_in[:, woff:woff + BVW])
                xa = pxe.tile([128, 2, W], f16, tag="xa", name="xa")
                nc.sync.dma_start(out=xa[:, :, :], in_=x_in[img, :, 0:2, :])
                xb = pxe.tile([128, 4, W], f16, tag="xb", name="xb")
                nc.sync.dma_start(out=xb[:, :, :], in_=x_in[img, :, 2:6, :])
                xc = pxe.tile([128, 4, W], f16, tag="xc", name="xc")
                nc.sync.dma_start(out=xc[:, :, :], in_=x_in[img, :, 6:NG, :])
                nc.scalar.dma_start(out=w_sb[:, BVW:WCOLS],
                                    in_=w_in[:, woff + BVW:woff + WCOLS])
                tiles[img] = (w_sb, xa, xb, xc)

            for img in range(IMG_PER_CORE):
                w_sb, xa, xb, xc = tiles[img]

                def win(g):
                    if g < 2:
                        return xa[:, g]
                    if g < 6:
                        return xb[:, g - 2]
                    return xc[:, g - 6]

                # ---- stage A.  One PSUM bank per window pair, v-major
                # layout [v, gi, n]; y' cols 256..269 of tpl get garbage
                # from g9's unwritten PSUM tail (never read by stage B).
                tpl = {cc: pt.tile([128, NV, 256], f16, tag=f"T{cc}",
                                   name=f"T{cc}")
                       for cc in range(8)}
                for gp in range(5):
                    if img == 0 and gp == 3:
                        # image-0's xc windows are still in flight (the DMA
                        # engine ramps from ~110 to ~440 GB/s over the first
                        # 10us); keep the PE hot instead of idling.
                        for _ in range(12):
                            nc.tensor.matmul(wu[:, :], lhsT=zt[:, 0:128],
                                             rhs=zt[:, :], start=True,
                                             stop=True)
                    for cc in range(8):
                        ps = psA.tile([128, 512], f32, tag="psA", name="ps")
                        r = ps[:, 0:486].rearrange(
                            "P (v gi n) -> P v gi n", v=NV, gi=2, n=GRP)
                        if gp < 4:
                            for gi in range(2):
                                g = 2 * gp + gi
                                rhs = (w_sb[:, 0:243] if g == 0
                                       else w_sb[:, 243:486])
                                nc.tensor.matmul(
                                    r[:, :, gi, :],
                                    lhsT=win(g)[:, 128 * cc:128 * (cc + 1)],
                                    rhs=rhs, start=True, stop=True)
                        else:
                            nc.tensor.matmul(
                                r[:, :, 0, :],
                                lhsT=win(8)[:, 128 * cc:128 * (cc + 1)],
                                rhs=w_sb[:, 243:486], start=True, stop=True)
                            nc.tensor.matmul(
                                r[:, :, 1, 0:13],
                                lhsT=win(9)[:, 128 * cc:128 * (cc + 1)],
                                rhs=w_sb[:, 486:603], start=True, stop=True)
                        src = ps[:, 0:486].rearrange(
                            "P (v gin) -> P v gin", v=NV, gin=2 * GRP)
                        if gp < 4:
                            copy(tpl[cc][:, :, 54 * gp:54 * gp + 54], src)
                        else:
                            # g9 only has 13 live columns: the live 40 of
                            # the 54 (gi, n) columns are contiguous in the
                            # v-major bank, so skip the unwritten tail.
                            copy(tpl[cc][:, :, 216:256], src[:, :, 0:40])

                # ---- stage B: accumulate all (tile, v) blocks into out PSUM
                for yc in range(2):
                    po = psB.tile([128, OUT], f32, tag="psB")
                    # zero + set PSUM written-bits via an all-zero matmul;
                    # streams the static zero tile so it can issue early.
                    nc.tensor.matmul(
                        po[:, :], lhsT=zt[:, 0:128], rhs=zt[:, :],
                        start=True, stop=False, skip_group_check=True)
                    for bi, (v, t, x0, x1, koff) in enumerate(BLOCKS):
                        nc.tensor.matmul(
                            po[:, x0:x1],
                            lhsT=tpl[t][:, v, 128 * yc:128 * yc + 128],
                            rhs=w_sb[:, BVW + koff:BVW + koff + (x1 - x0)],
                            start=False, stop=(bi == len(BLOCKS) - 1),
                            skip_group_check=True)
                    ot = pout.tile([128, OUT], f16, tag="ot")
                    copy(ot[:, :], po[:, :])
                    nc.sync.dma_start(
                        out=out_t[img, 128 * yc:128 * (yc + 1), :],
                        in_=ot[:, :])
    _split_multi_waits(nc)
    return nc


def _get_program():
    global _PROGRAM
    if _PROGRAM is None:
        _PROGRAM = _build_program()
    return _PROGRAM


# ------------------------------------------------------------------ kernel
def _window_image(img16):
    """[NG, 128, W] f16 pre-extracted row windows; out-of-image partitions
    get arbitrary finite rows (their BV weights are exactly zero)."""
    xw = np.empty((NG, 128, W), np.float16)
    for g in range(NG):
        wb = _wb(g)
        r0, r1 = max(0, wb), min(H, wb + 128)
        p0, pn = r0 - wb, r1 - r0
        xw[g, p0:p0 + pn] = img16[r0:r1]
        if p0 > 0:
            xw[g, 0:p0] = img16[0:p0]
        if p0 + pn < 128:
            xw[g, p0 + pn:] = img16[0:128 - (p0 + pn)]
    return xw


def _make_in_maps(outputs, mtf, r, c):
    outputs = np.ascontiguousarray(outputs, np.float32)
    mtf64 = np.asarray(mtf, np.float64)
    in_maps = []
    for core in range(N_CORES):
        xs = np.empty((IMG_PER_CORE, 128, NG, W), np.float16)
        w = np.empty((128, IMG_PER_CORE, WCOLS), np.float16)
        for i in range(IMG_PER_CORE):
            p = core * IMG_PER_CORE + i
            b, ch = divmod(p, 8)
            xs[i] = _window_image(
                outputs[b, ch].astype(np.float16)).transpose(1, 0, 2)
            BV = build_BV(mtf64[:, :, ch], int(r[b, ch]))   # [NG,9,128,GRP]
            w[:, i, 0:243] = BV[0].transpose(1, 0, 2).reshape(
                128, 243).astype(np.float16)
            w[:, i, 243:486] = BV[1].transpose(1, 0, 2).reshape(
                128, 243).astype(np.float16)
            w[:, i, 486:603] = BV[9][:, :, :13].transpose(1, 0, 2).reshape(
                128, 117).astype(np.float16)
            w[:, i, BVW:] = build_KHV(int(c[b, ch])).astype(np.float16)
        in_maps.append({
            "x": xs,
            "w": np.ascontiguousarray(w.reshape(128, -1)),
        })
    return in_maps


def run(outputs, mtf, r, c, trace=False, trace_cores=None):
    from concourse.bass_utils import run_bass_kernel_spmd

    nc = _get_program()
    in_maps = _make_in_maps(outputs, mtf, r, c)
    res = run_bass_kernel_spmd(nc, in_maps, core_ids=list(range(N_CORES)),
                               trace=trace, trace_cores=trace_cores)
    full = np.empty((4, 8, OUT, OUT), np.float32)
    for core in range(N_CORES):
        o = np.asarray(res.results[core]["out"])
        for i in range(IMG_PER_CORE):
            p = core * IMG_PER_CORE + i
            b, ch = divmod(p, 8)
            full[b, ch] = o[i].astype(np.float32)
    return full, res


def kernel(outputs, mtf, r, c):
    full, _ = run(outputs, mtf, r, c)
    return full
